# revision 1
# baseline (speedup 1.0000x reference)
"""Trainium2 Bass kernel for nn_KernelActivation (k=2 patch permutation).

The reference op is a pure element permutation of x:(16,64,224,224) fp32:
  view x as (b, i, p, j, q, w) = (16, 32, 2, 112, 2, 224)
  out  is  (b, i, j, w, p, q) flattened back to (16, 64, 224, 224)
i.e. out[b, i, j, w, p, q] = x[b, i, p, j, q, w].

Sharding: batch dim across 8 cores (2 batch elements per core), fully local.

Per-core program (64 blocks = 2 batches x 32 i):
  - load  x[b,i] into SBUF tile [j=112 partitions, (p,q,w)=896 els]
    (DMA reads two contiguous 200KB regions, 1792B descriptors)
  - 4 strided copies (one per (p,q)) interleave (p,q,w) -> (w,p,q)
    split across Vector and Scalar engines
  - store out[b,i] from SBUF tile [j=112, (w,p,q)=896] as one fully
    contiguous 401KB write
"""

import os
import sys

import numpy as np

sys.path.insert(0, "/opt/trn_rl_repo")

import concourse.bass as bass
import concourse.bacc as bacc
import concourse.mybir as mybir
import concourse.tile as tile
from concourse.bass_utils import run_bass_kernel_spmd

N_CORES = 8
B, C, H, W = 16, 64, 224, 224
K = 2
BPC = B // N_CORES  # batches per core
I, J = C // K, H // K  # 32, 112

_nc_cache = {}


def _build_program(reps: int = 1):
    key = ("nc", reps)
    if key in _nc_cache:
        return _nc_cache[key]

    nc = bacc.Bacc("TRN2", target_bir_lowering=False, debug=False)
    X = nc.dram_tensor("x", [BPC, C, H, W], mybir.dt.float32, kind="ExternalInput").ap()
    O = nc.dram_tensor(
        "out", [BPC, C, H, W], mybir.dt.float32, kind="ExternalOutput"
    ).ap()

    # x viewed as (b, i, p, j, q, w)
    Xv = X.rearrange("b (i p) (j q) w -> b i p j q w", p=K, q=K)
    # out viewed as (b, i, flat(j,w,p,q))
    Ov = O.rearrange("b (i x) h w -> b i (x h w)", x=K)

    FREE = K * K * W  # 896 elements per partition

    with tile.TileContext(nc) as tc:
        with (
            tc.tile_pool(name="tin", bufs=4) as tin_pool,
            tc.tile_pool(name="tout", bufs=4) as tout_pool,
        ):
            for _rep in range(reps):
                for b in range(BPC):
                    for i in range(I):
                        blk = b * I + i
                        # ---- load: [j, (p, q, w)] <- x[b, i, p, j, q, w]
                        t_in = tin_pool.tile([J, FREE], mybir.dt.float32)
                        src = Xv[b, i].transpose([1, 0, 2, 3])  # (j, p, q, w)
                        nc.sync.dma_start(out=t_in[:], in_=src)

                        # ---- shuffle: single 4D-AP copy, free (w,p,q) <- (p,q,w)
                        # engine alternates per block so each tile has ONE
                        # compute-engine reader/writer (keeps DMA waits <= 2)
                        t_out = tout_pool.tile([J, FREE], mybir.dt.float32)
                        dstv = t_out.rearrange("j (w p q) -> j w p q", w=W, p=K, q=K)
                        srcv = t_in.rearrange("j (p q w) -> j w p q", p=K, q=K, w=W)
                        if blk % 2 == 0:
                            nc.vector.tensor_copy(out=dstv, in_=srcv)
                        else:
                            nc.scalar.copy(out=dstv, in_=srcv)

                        # ---- store: contiguous 401KB
                        nc.scalar.dma_start(out=Ov[b, i], in_=t_out[:])

    nc.compile()
    _nc_cache[key] = nc
    return nc


def _legalize_dma_waits(nc):
    """Walrus codegen allows only ONE inline sync-wait on a DMACopy.

    Tile (optimize_sems disabled) emits redundant waits: e.g. a load waits
    both on the slot's reader (engine sem) AND on the slot's previous
    writer DMA (DMAHW sem) even though the reader already waited on that
    writer. Engines execute in order, so a wait on the k-th increment of
    engine sem E is implied to cover every wait any E-instruction up to
    that point carried. Drop DMA waits that are transitively implied;
    assert <=1 wait remains.
    """
    insts = []
    for blk in nc.m.functions[0].blocks:
        insts.extend(blk.instructions)

    # engine sem name -> list of (cumulative value after update, inherited
    # waits dict at that instruction)
    sem_hist = {}
    # running per-engine inherited waits: engine -> {sem_name: max value}
    eng_inherited = {}

    def implied(retained, sem, val):
        # retained: list of (sem_name, value) kept on the DMA
        for rs, rv in retained:
            hist = sem_hist.get(rs)
            if not hist:
                continue
            # find instruction whose update made rs reach rv
            for cum, inh in hist:
                if cum >= rv:
                    if inh.get(sem, 0) >= val:
                        return True
                    break
        return False

    for inst in insts:
        si = inst.sync_info
        eng = str(inst.engine)
        inh = eng_inherited.setdefault(eng, {})
        waits = list(si.on_wait) if si and si.on_wait else []

        if type(inst).__name__ == "InstDMACopy" and len(waits) > 1:
            # prefer keeping engine (compute) waits; try to drop the rest
            keep = []
            for w in sorted(
                waits, key=lambda w: ("DMAHW" in w.ant_name, w.ant_name)
            ):
                retained = [(k.ant_name, k.wait_value) for k in keep]
                if keep and implied(retained, w.ant_name, w.wait_value):
                    continue
                keep.append(w)
            if len(keep) > 1:
                # Force down to 1: engine (compute) waits guard WAR on SBUF
                # slots and must stay; DMAHW waits here guard WAW on
                # disjoint DRAM regions (Tile tracks DRAM coarsely) and are
                # safe to drop. CoreSim's race detector is the gate.
                eng_waits = [w for w in keep if "DMAHW" not in w.ant_name]
                assert len(eng_waits) <= 1, (
                    f"{inst.name}: two compute-engine waits survive: "
                    f"{[(w.ant_name, w.wait_value) for w in keep]}"
                )
                keep = eng_waits or keep[:1]
            si.on_wait = keep
            waits = keep

        # update running inherited-wait state for this engine
        for w in waits:
            if inh.get(w.ant_name, 0) < w.wait_value:
                inh[w.ant_name] = w.wait_value

        # record sem updates with a snapshot of inherited waits
        if si and si.on_update:
            for u in si.on_update:
                hist = sem_hist.setdefault(u.ant_name, [])
                prev = hist[-1][0] if hist else 0
                hist.append((prev + u.update_value, dict(inh)))


def kernel(x: np.ndarray) -> np.ndarray:
    x = np.ascontiguousarray(np.asarray(x, dtype=np.float32))
    assert x.shape == (B, C, H, W), x.shape

    nc = _build_program()
    in_maps = [{"x": x[c * BPC : (c + 1) * BPC]} for c in range(N_CORES)]
    trace = bool(int(os.environ.get("KERNEL_TRACE", "0")))
    res = run_bass_kernel_spmd(nc, in_maps, list(range(N_CORES)), trace=trace)
    if trace:
        _nc_cache["last_results"] = res
    out = np.concatenate([res.results[c]["out"] for c in range(N_CORES)], axis=0)
    return out

